# revision 1
# baseline (speedup 1.0000x reference)
"""Trainium2 Bass kernel for nn_CrossAttentionFusion.

Reference computation (B=16384, img_dim=2048, tab_dim=128, E=256):
    img_p   = img_embed @ Wi.T + bi                      (B, E)
    tab_p   = tab_embed @ Wt.T + bt                      (B, E)
    img_att = LN(tab_p @ Wc_img.T + bc_img + img_p)      Wc_img = out_w_img @ Wv_img
    tab_att = LN(img_p @ Wc_tab.T + bc_tab + tab_p)
    out     = concat([img_att, tab_att], -1)             (B, 2E)

Host-side algebra (exact):
  * The two 256x256 attention matmuls fold into one:  Wc = out_w @ in_w[2E:].
  * The img-attention path folds all the way to tab_embed:
        tab_p @ Wc_img.T = tab_embed @ (Wc_img @ Wt).T + (Wc_img @ bt)
    so it shares the stationary x_tab operand with the tab projection.
  * All biases collapse into one per-side vector added before LN.

Device plan (pure data parallel, batch sharded 8 ways, weights replicated):
  batch-major layout everywhere; x is passed transposed from host so the
  contraction dim lands on SBUF partitions.  Per 128-row b-tile:
    psum_A (128,256) = sum_k xiT[k,b].T @ WiT[k]          img_p (16 matmuls)
    psum_B (128,512) = xtT[b].T @ [WtT | WfoldT_img]      tab_p | img_att_pre
    psum_T (128,256) = PE-transpose(img_p)                img_p.T for next mm
    psum_C (128,256) = sum_et imgpT[et].T @ WcT_tab[et]   tab_att_pre
    s_img = img_p + img_att_pre (+bias);  s_tab = tab_p + tab_att_pre (+bias)
    LayerNorm both via bn_stats/bn_aggr, write (128,512) out tile.

Scheduling (the kernel is HBM-stream-bound; DMA transfers execute serially
at ~343 GB/s on the 16 SDMA engines):
  * xi is loaded in 1 MB chunk tiles so dependency tracking releases the
    first matmuls after one chunk lands; slab widths taper (...384,128) so
    the final output is gated by a small trailing load.
  * loads ride the SP HWDGE ring; weights/xt/stores ride the ACT ring so a
    result-dependent store never blocks loads (SP ring is strict FIFO).
  * global 3-stage software pipeline A/B(j) | T(j-1) | C(j-2) keeps the PE
    stream dense; ~3.4 us of bf16 identity matmuls during the DMA lead-in
    open the HAM clock gate before real work.
  * the PE stream stays single-dtype (f32r transposes incl. identity):
    mixing fp32 2-pass transposes with f32r matmuls intermittently faulted
    the exec unit on HW.
  * per-slab batched LN scale (one ACT Sqrt table visit per slab).

Measured on trn2 (8 cores, steady-state per-iteration via For_i delta):
  f32r: ~78-85 us, rel err 1.5e-04;  bf16: ~50-59 us, rel err 2.4e-03.
"""

import json
import os

import numpy as np

E = 256
IMG_DIM = 2048
TAB_DIM = 128
B_FULL = 16384
N_CORES = 8
B_LOC = B_FULL // N_CORES  # 2048
P = 128
KT = IMG_DIM // P  # 16 k-tiles for the img contraction
SLAB = 512  # b-columns fetched per DMA slab
NSLAB = B_LOC // SLAB  # 4
TPS = SLAB // P  # 4 b-tiles per slab
EPS = 1e-5

# matmul/data mode: "bf16" (bf16 HBM data + matmuls, ~50 us/core, rel err
# ~2.4e-3), "f32r" (fp32 HBM data, fp32r matmuls, ~77 us/core, rel err
# ~1.5e-4), "f32" (plain fp32 matmuls, slowest, exact-est)
MM_MODE = os.environ.get("KERNEL_MM_MODE", "f32r")

_cache: dict = {}


def _split_multi_waits(bir_bytes: bytes) -> bytes:
    """Work around this walrus build's 1-sync-wait-per-instruction limit.

    Any BIR instruction with >1 `on_wait` fails codegen ("Too many sync wait
    commands").  Hoist all but the last wait onto same-engine EventSemaphore
    instructions inserted immediately before; engines run their stream in
    order, so sequential sem waits are equivalent.
    """
    m = json.loads(bir_bytes)
    for f in m["functions"]:
        for b in f["blocks"]:
            out = []
            for ins in b["instructions"]:
                si = ins.get("sync_info")
                waits = (si or {}).get("on_wait") or []
                if len(waits) > 1:
                    for i, extra in enumerate(waits[:-1]):
                        out.append(
                            {
                                "debug": ins.get("debug", 0),
                                "engine": ins["engine"],
                                "ins": [],
                                "outs": [],
                                "name": f"{ins['name']}-ws{i}",
                                "opcode": "EventSemaphore",
                                "sync_info": {"on_update": [], "on_wait": [extra]},
                            }
                        )
                    si["on_wait"] = [waits[-1]]
                out.append(ins)
            b["instructions"] = out
    return json.dumps(m).encode()


def _build_module(use_bias: bool, use_gb: bool, mode: str, reps: int = 1):
    """reps>1 wraps the body in a hardware loop — benchmarking only."""
    import contextlib

    import concourse.bass as bass
    import concourse.mybir as mybir
    import concourse.tile as tile
    from concourse.masks import make_identity

    f32 = mybir.dt.float32
    # dtype of x/weight data in DRAM and SBUF (the matmul operand dtype)
    xdt = {
        "f32r": mybir.dt.float32r,
        "bf16": mybir.dt.bfloat16,
        "f32": f32,
    }[mode]
    # transpose path runs in the matmul dtype: a single-dtype PE stream
    # avoids the fp32 2-pass (LO/HI) transpose interleaving with f32r
    # matmuls, which intermittently faults the exec unit on HW
    tdt = xdt if mode != "f32" else f32

    nc = bass.Bass()

    xiT = nc.dram_tensor("xiT", [IMG_DIM, B_LOC], xdt, kind="ExternalInput")
    xtT = nc.dram_tensor("xtT", [TAB_DIM, B_LOC], xdt, kind="ExternalInput")
    wiT = nc.dram_tensor("wiT", [IMG_DIM, E], xdt, kind="ExternalInput")
    wcat = nc.dram_tensor("wcat", [TAB_DIM, 2 * E], xdt, kind="ExternalInput")
    wctT = nc.dram_tensor("wctT", [E, E], xdt, kind="ExternalInput")
    out = nc.dram_tensor("out", [B_LOC, 2 * E], f32, kind="ExternalOutput")
    if use_bias:
        bias_d = nc.dram_tensor("bias", [2 * E], f32, kind="ExternalInput")
    if use_gb:
        lng_d = nc.dram_tensor("lng", [E], f32, kind="ExternalInput")
        lnb_d = nc.dram_tensor("lnb", [E], f32, kind="ExternalInput")

    sub = mybir.AluOpType.subtract
    mult = mybir.AluOpType.mult

    with tile.TileContext(nc) as tc:
        with (
            tc.tile_pool(name="consts", bufs=1) as consts,
            tc.tile_pool(name="xi_pool", bufs=12) as xi_pool,
            tc.tile_pool(name="xt_pool", bufs=3) as xt_pool,
            tc.tile_pool(name="work", bufs=3) as work,
            tc.tile_pool(name="outp", bufs=3) as outp,
            tc.tile_pool(name="psA", bufs=2, space="PSUM") as psA,
            tc.tile_pool(name="psB", bufs=2, space="PSUM") as psB,
            tc.tile_pool(name="psC", bufs=2, space="PSUM") as psC,
            tc.tile_pool(name="psT", bufs=2, space="PSUM") as psT,
        ):
            # ---- constants ----
            # wi as 4 separate chunk tiles: matmul k waits only its chunk
            KC = KT // 4
            wiT_r = wiT.rearrange("(t p) e -> p t e", p=P)
            wi_cs = []
            for c in range(4):
                w = consts.tile([P, KC, E], xdt, name=f"wi_c{c}")
                nc.scalar.dma_start(
                    out=w, in_=wiT_r[:, c * KC : (c + 1) * KC, :]
                )
                wi_cs.append(w)
            wcat_sb = consts.tile([P, 2 * E], xdt)
            nc.scalar.dma_start(out=wcat_sb, in_=wcat.ap())
            wct_sb = consts.tile([P, 2, E], xdt)
            nc.scalar.dma_start(out=wct_sb, in_=wctT.rearrange("(t p) e -> p t e", p=P))
            ident_f = consts.tile([P, P], f32)
            make_identity(nc, ident_f)
            # rounding copy to the matmul dtype (f32r operands must be
            # produced as f32r; ACT copy is the rounding op)
            if tdt == f32:
                ident = ident_f
            else:
                ident = consts.tile([P, P], tdt)
                nc.scalar.copy(ident, ident_f)
            eps_col = consts.tile([P, 1], f32)
            nc.vector.memset(eps_col, EPS)

            # PE warm-up: ~4 us of dummy bf16 matmuls during the DMA
            # lead-in so the HAM clock gate opens (1.2 -> 2.4 GHz) before
            # real work; strictly precedes all f32r work on the PE.
            # ident_w is built directly in bf16 so the warmup doesn't wait
            # on the fp32 identity + ACT copy chain.
            ident_w = consts.tile([P, P], mybir.dt.bfloat16)
            make_identity(nc, ident_w)
            warm_ps = psA.tile([P, P], f32, name="warm_ps", tag="pA")
            for _ in range(32):
                nc.tensor.matmul(warm_ps, lhsT=ident_w, rhs=ident_w,
                                 start=True, stop=True)
            if use_bias:
                bias_sb = consts.tile([P, 2 * E], f32)
                nc.sync.dma_start(out=bias_sb, in_=bias_d.ap().to_broadcast((P, 2 * E)))
            if use_gb:
                lng_sb = consts.tile([P, E], f32)
                nc.sync.dma_start(out=lng_sb, in_=lng_d.ap().to_broadcast((P, E)))
                lnb_sb = consts.tile([P, E], f32)
                nc.sync.dma_start(out=lnb_sb, in_=lnb_d.ap().to_broadcast((P, E)))

            xiT_r = xiT.rearrange("(t p) b -> p t b", p=P)
            out_r = out.rearrange("(t p) e -> p t e", p=P)

            # slab widths taper at the end so the final output is gated by a
            # small trailing load, not a full 4 MB slab
            SLAB_W = [512, 512, 512, 384, 128]
            assert sum(SLAB_W) == B_LOC
            slab_b0 = [sum(SLAB_W[:i]) for i in range(len(SLAB_W))]
            # global b-tile table: (slab, j-within-slab, global row block)
            btiles = []
            for s, w in enumerate(SLAB_W):
                for j in range(w // P):
                    btiles.append((s, j))
            NB = len(btiles)
            last_jj_of_slab = {s: max(i for i, (s2, _) in enumerate(btiles)
                                      if s2 == s) for s in range(len(SLAB_W))}

            loop_cm = tc.For_i(0, reps, 1) if reps > 1 else contextlib.nullcontext()
            with loop_cm:
                # per-slab state dicts (python-side bookkeeping only)
                chunks: dict = {}
                xts: dict = {}
                slab_bufs: dict = {}
                sts: dict = {}
                imgpTs: dict = {}

                # chunks per slab: fine at startup (dependency release) and
                # in the tapered tail; 2 MB chunks mid-stream where larger
                # transfers run closer to HBM line rate
                NCH = [4, 4, 4, 4, 4]

                def load_slab(s):
                    w = SLAB_W[s]
                    kc = KT // NCH[s]
                    bs = slice(slab_b0[s], slab_b0[s] + w)
                    # separate chunk tiles so dependency tracking lets the
                    # first matmuls start after one chunk lands, not the slab
                    for c in range(NCH[s]):
                        t = xi_pool.tile([P, kc, w], xdt, tag="xi",
                                         name=f"xi{s}_{c}")
                        ks = slice(c * kc, (c + 1) * kc)
                        nc.sync.dma_start(out=t, in_=xiT_r[:, ks, bs])
                        chunks[(s, c)] = t
                    xt = xt_pool.tile([P, w], xdt, tag="xt", name=f"xt{s}")
                    nc.scalar.dma_start(out=xt, in_=xtT[:, bs])
                    xts[s] = xt
                    s_all = work.tile([P, TPS, 2, E], f32, tag="s_all",
                                      name=f"s_all{s}")
                    mv_all = work.tile([P, TPS, 2, 2], f32, tag="mv_all",
                                       name=f"mv_all{s}")
                    slab_bufs[s] = (s_all, mv_all)

                def stage1(jj):
                    """projection matmuls + PSUM->SBUF copies for b-tile jj"""
                    s, j = btiles[jj]
                    bcol = slice(j * P, (j + 1) * P)
                    pA = psA.tile([P, E], f32, tag="pA", name=f"pA{jj}")
                    kc = KT // NCH[s]
                    for k in range(KT):
                        nc.tensor.matmul(
                            pA,
                            lhsT=chunks[(s, k // kc)][:, k % kc, bcol],
                            rhs=wi_cs[k // KC][:, k % KC, :],
                            start=(k == 0),
                            stop=(k == KT - 1),
                        )
                    pB = psB.tile([P, 2 * E], f32, tag="pB", name=f"pB{jj}")
                    nc.tensor.matmul(
                        pB, lhsT=xts[s][:, bcol], rhs=wcat_sb,
                        start=True, stop=True,
                    )
                    # img_p copy (ACT) doubles as transpose input and
                    # residual operand; tab_p copy on DVE
                    imgp_s = work.tile([P, E], tdt, tag="imgp", name=f"imgp{jj}")
                    nc.scalar.copy(imgp_s, pA)
                    tabp_s = work.tile([P, E], f32, tag="tabp", name=f"tabp{jj}")
                    nc.vector.tensor_copy(tabp_s, pB[:, 0:E])
                    sts[jj] = (pA, pB, imgp_s, tabp_s)

                def stageT(jj):
                    """transpose img_p, img-side residual sum + LN stats"""
                    s, j = btiles[jj]
                    s_all, mv_all = slab_bufs[s]
                    pA, pB, imgp_s, tabp_s = sts[jj]
                    pT = psT.tile([P, E], tdt, tag="pT", name=f"pT{jj}")
                    for et in range(2):
                        nc.tensor.transpose(
                            pT[:, et * P : (et + 1) * P],
                            imgp_s[:, et * P : (et + 1) * P],
                            ident,
                        )
                    # rounding/casting copy to the matmul operand dtype
                    imgpT = work.tile([P, E], xdt, tag="imgpT", name=f"imgpT{jj}")
                    nc.scalar.copy(imgpT, pT)
                    s_img = s_all[:, j, 0, :]
                    imgp_f = (imgp_s.bitcast(f32)
                              if imgp_s.dtype == mybir.dt.float32r else imgp_s)
                    nc.vector.tensor_add(s_img, imgp_f, pB[:, E : 2 * E])
                    if use_bias:
                        nc.gpsimd.tensor_add(s_img, s_img, bias_sb[:, 0:E])
                    stats = work.tile([P, 6], f32, tag="st0")
                    nc.vector.bn_stats(out=stats, in_=s_all[:, j, 0, :])
                    nc.vector.bn_aggr(out=mv_all[:, j, 0, :], in_=stats)
                    imgpTs[jj] = imgpT

                def stageC(jj):
                    """tab-attention matmul, tab-side sum + LN stats"""
                    s, j = btiles[jj]
                    s_all, mv_all = slab_bufs[s]
                    pA, pB, imgp_s, tabp_s = sts[jj]
                    imgpT = imgpTs[jj]
                    pC = psC.tile([P, E], f32, tag="pC", name=f"pC{jj}")
                    for et in range(2):
                        nc.tensor.matmul(
                            pC,
                            lhsT=imgpT[:, et * P : (et + 1) * P],
                            rhs=wct_sb[:, et, :],
                            start=(et == 0),
                            stop=(et == 1),
                        )
                    s_tab = s_all[:, j, 1, :]
                    nc.vector.tensor_add(s_tab, tabp_s, pC)
                    if use_bias:
                        nc.gpsimd.tensor_add(s_tab, s_tab, bias_sb[:, E : 2 * E])
                    stats = work.tile([P, 6], f32, tag="st1")
                    nc.vector.bn_stats(out=stats, in_=s_all[:, j, 1, :])
                    nc.vector.bn_aggr(out=mv_all[:, j, 1, :], in_=stats)

                def epilogue(s):
                    """batched LN scale + apply + output DMA for slab s"""
                    tps_s = SLAB_W[s] // P
                    s_all, mv_all = slab_bufs[s]
                    sd_all = work.tile([P, TPS, 2], f32, tag="sd_all")
                    nc.scalar.activation(
                        out=sd_all[:, 0:tps_s, :], in_=mv_all[:, 0:tps_s, :, 1],
                        func=mybir.ActivationFunctionType.Sqrt,
                        bias=eps_col, scale=1.0,
                    )
                    rstd_all = work.tile([P, TPS, 2], f32, tag="rstd_all")
                    nc.vector.reciprocal(rstd_all[:, 0:tps_s, :],
                                         sd_all[:, 0:tps_s, :])
                    o_slab = outp.tile([P, TPS, 2 * E], f32, tag="o")
                    for j in range(tps_s):
                        for side in (0, 1):
                            o_slice = o_slab[:, j, side * E : (side + 1) * E]
                            dst = o_slice
                            if use_gb:
                                dst = work.tile([P, E], f32, tag=f"n{side}")
                            nc.vector.tensor_scalar(
                                out=dst, in0=s_all[:, j, side, :],
                                scalar1=mv_all[:, j, side, 0:1],
                                scalar2=rstd_all[:, j, side : side + 1],
                                op0=sub, op1=mult,
                            )
                            if use_gb:
                                scaled = work.tile([P, E], f32, tag=f"sc{side}")
                                nc.gpsimd.tensor_mul(scaled, dst, lng_sb)
                                nc.gpsimd.tensor_add(o_slice, scaled, lnb_sb)
                    # output DMA on the ACT HWDGE ring so a result-dependent
                    # store never blocks input loads queued on the SP ring
                    t0 = slab_b0[s] // P
                    nc.scalar.dma_start(
                        out=out_r[:, t0 : t0 + tps_s, :],
                        in_=o_slab[:, 0:tps_s, :],
                    )

                # global 3-stage software pipeline over all b-tiles: the
                # PE stream interleaves A/B(jj) | T(jj-1) | C(jj-2) so every
                # ACT copy has a full projection block of time to land, and
                # slab epilogues are emitted only after their stats complete.
                # The last two (tapered) slabs run T/C immediately so the
                # final LN chain starts as early as possible.
                TAIL_JJ = last_jj_of_slab[len(SLAB_W) - 3] + 1

                def maybe_epilogue(x):
                    s2 = btiles[x][0]
                    if x == last_jj_of_slab[s2]:
                        epilogue(s2)

                loaded = set()
                for jj in range(NB):
                    s_cur = btiles[jj][0]
                    if s_cur not in loaded:
                        loaded.add(s_cur)
                        load_slab(s_cur)
                    stage1(jj)
                    if jj < TAIL_JJ:
                        if jj >= 1:
                            stageT(jj - 1)
                        if jj >= 2:
                            stageC(jj - 2)
                            maybe_epilogue(jj - 2)
                    else:
                        if jj == TAIL_JJ:
                            stageT(jj - 1)
                            stageC(jj - 2)
                            maybe_epilogue(jj - 2)
                            stageC(jj - 1)
                            maybe_epilogue(jj - 1)
                        stageT(jj)
                        stageC(jj)
                        maybe_epilogue(jj)

    return nc


def _prep_inputs(inputs: dict, mode: str):
    """Host-side shard + weight folding. Returns (in_maps, use_bias, use_gb)."""
    import ml_dtypes

    f = lambda k: np.asarray(inputs[k], dtype=np.float64)
    Wi, bi = f("Wi"), f("bi")
    Wt, bt = f("Wt"), f("bt")
    Wc_img = f("out_w_img") @ f("in_w_img")[2 * E :]
    bc_img = f("out_w_img") @ f("in_b_img")[2 * E :] + f("out_b_img")
    Wc_tab = f("out_w_tab") @ f("in_w_tab")[2 * E :]
    bc_tab = f("out_w_tab") @ f("in_b_tab")[2 * E :] + f("out_b_tab")

    Wfold_img = Wc_img @ Wt  # (E, TAB_DIM)
    bias_img = bi + Wc_img @ bt + bc_img
    bias_tab = bt + Wc_tab @ bi + bc_tab
    bias = np.concatenate([bias_img, bias_tab]).astype(np.float32)

    lng = np.asarray(inputs["ln_g"], dtype=np.float32)
    lnb = np.asarray(inputs["ln_b"], dtype=np.float32)
    use_bias = bool(np.any(bias != 0.0))
    use_gb = bool(np.any(lng != 1.0) or np.any(lnb != 0.0))

    xdt = ml_dtypes.bfloat16 if mode == "bf16" else np.float32
    wiT = np.ascontiguousarray(Wi.T).astype(xdt)
    wcat = np.concatenate([Wt.T, Wfold_img.T], axis=1).astype(xdt)  # (128, 512)
    wctT = np.ascontiguousarray(Wc_tab.T).astype(xdt)

    xi = np.asarray(inputs["img_embed"], dtype=np.float32)
    xt = np.asarray(inputs["tab_embed"], dtype=np.float32)
    xiT = np.ascontiguousarray(xi.T).astype(xdt)  # (IMG_DIM, B)
    xtT = np.ascontiguousarray(xt.T).astype(xdt)  # (TAB_DIM, B)

    in_maps = []
    for c in range(N_CORES):
        bs = slice(c * B_LOC, (c + 1) * B_LOC)
        m = {
            "xiT": np.ascontiguousarray(xiT[:, bs]),
            "xtT": np.ascontiguousarray(xtT[:, bs]),
            "wiT": wiT,
            "wcat": wcat,
            "wctT": wctT,
        }
        if use_bias:
            m["bias"] = bias
        if use_gb:
            m["lng"] = lng
            m["lnb"] = lnb
        in_maps.append(m)
    return in_maps, use_bias, use_gb


def _kernel_impl(inputs: dict, trace: bool):
    from concourse.bass_utils import run_bass_kernel_spmd

    mode = MM_MODE
    in_maps, use_bias, use_gb = _prep_inputs(inputs, mode)
    key = (use_bias, use_gb, mode)
    if key not in _cache:
        nc = _build_module(use_bias, use_gb, mode)
        # work around this walrus build's 1-wait-per-instruction limit
        orig = nc.to_json_bytes
        nc.to_json_bytes = lambda: _split_multi_waits(orig())
        _cache[key] = nc
    nc = _cache[key]

    try:
        res = run_bass_kernel_spmd(
            nc,
            in_maps,
            core_ids=list(range(N_CORES)),
            trace=trace,
            trace_cores=[0] if trace else None,
        )
    except ModuleNotFoundError:
        # no NTFF profile hook in this container; run without trace
        res = run_bass_kernel_spmd(nc, in_maps, core_ids=list(range(N_CORES)))
    out = np.concatenate([r["out"] for r in res.results], axis=0)
    return out, res


def kernel(**inputs) -> np.ndarray:
    out, _ = _kernel_impl(inputs, trace=False)
    return out


def kernel_traced(**inputs):
    return _kernel_impl(inputs, trace=True)

